# revision 36
# baseline (speedup 1.0000x reference)
"""Trainium2 Bass kernel for nn_AMPGCN (embedding_lookup + 3x BatchNorm/ReLU +
mean-pool + linear + log_softmax), distributed over 8 NeuronCores.

Algorithm
---------
Every activation x[n, (s,d)] through the BN/ReLU chain depends only on
(v=idx[n,s], s, d), because BatchNorm is a per-channel affine map. So instead
of materializing the [20000, 15360] activation tensor (1.2 GB), each core:

  1. builds per-position histograms CNT[s, v] of the sampled indices
     (one-hot outer-product matmuls on the TensorEngine),
  2. runs the BN chain on tiny per-position tables T_s [1433, 768]
     (weighted sums with CNT give the exact batch statistics),
  3. collapses the linear head into per-position tables L_s = T_s @ W.T / S,
  4. all-gathers the L tables and gathers 7-float rows per (node, s)
     via dma_gather, sums over s, and applies log_softmax.

Work is sharded over the position axis s (20 positions over 8 cores, 3 waves),
the final gather + softmax over the node axis (2500 nodes/core).
"""
import sys

if "/opt/trn_rl_repo" not in sys.path:
    sys.path.insert(0, "/opt/trn_rl_repo")

import numpy as np
import ml_dtypes

import concourse.bass as bass
import concourse.tile as tile
from concourse import bacc, mybir
from concourse.bass import AP

# ---------------------------------------------------------------- constants
N, S, D, V, K = 20000, 20, 768, 1433, 7
EPS = 1e-5
NCORES = 8
P = 128
NLOC = N // NCORES            # 2500 nodes per core
NB = 20                       # node tiles per core (2560 padded)
NPAD = NB * P                 # 2560
TD = D // P                   # 6 d-tiles
SH = 3                        # s-slots per core (cores 4-7: slot 2 is dummy)
TN = 157                      # hist row-tiles per slot (ceil(20000/128))
NH = TN * P                   # 20096 padded hist rows
VA = 12                       # a = idx // 128 in [0, 12)
VPADD = 1536                  # VA * 128, padded vocab per slot in cnt scratch
GB = 8                        # hist batch size (tiles per vector instr)
ROWS_L = NCORES * V           # 11464 rows per gathered-wave table
EPITCH = 64                   # gather row pitch (64 f32 = 256B)
F32 = mybir.dt.float32
BF16 = mybir.dt.bfloat16
I16 = mybir.dt.int16

# wave t gathers positions s = 8t + c (c = contributing core); widths:
WAVE_W = [8, 8, 4]
WAVE_NBSPAN = [10, 10, 20]    # node-tile span per gather chunk
WAVE_CHUNKS = [2, 2, 1]
HOFF_COLS = NB * sum(WAVE_W)  # 400 gather offsets per partition


def _build(level=5):
    """Build + compile the SPMD Bass graph (identical on all 8 cores).

    level: pipeline prefix for hardware bisection
      1 = loads + histogram, 2 = +BN chain +L, 3 = +publish +collective,
      4 = +repad, 5 = full (gather + softmax).
    """
    nc = bacc.Bacc("TRN2", target_bir_lowering=False, debug=False,
                   num_devices=NCORES)

    # ------------------------------------------------------------- params
    emb_t = nc.dram_tensor("emb_t", [D, V], F32, kind="ExternalInput")
    # BN affine params, one [P, TD] block per (stage, slot)
    gvec = nc.dram_tensor("gvec", [3 * SH, P, TD], F32, kind="ExternalInput")
    bvec = nc.dram_tensor("bvec", [3 * SH, P, TD], F32, kind="ExternalInput")
    linw_t = nc.dram_tensor("linw_t", [P, TD, K], F32, kind="ExternalInput")
    linb_r = nc.dram_tensor("linb_r", [P, K], F32, kind="ExternalInput")
    hist_b = nc.dram_tensor("hist_b", [P, SH * TN], BF16, kind="ExternalInput")
    hist_a = nc.dram_tensor("hist_a", [P, SH * TN], BF16, kind="ExternalInput")
    iota128 = nc.dram_tensor("iota128", [P, P], BF16, kind="ExternalInput")
    arow = nc.dram_tensor("arow", [P, VA], BF16, kind="ExternalInput")
    hoff = nc.dram_tensor("hoff", [P, HOFF_COLS], mybir.dt.int32,
                          kind="ExternalInput")
    out_ext = nc.dram_tensor("out", [NLOC, K], F32, kind="ExternalOutput")
    debug = level == 5.1
    if debug:
        dbg_cnt = nc.dram_tensor("dbg_cnt", [SH, VPADD], F32,
                                 kind="ExternalOutput")
        dbg_l = nc.dram_tensor("dbg_l", [SH, ROWS_L * K], F32,
                               kind="ExternalOutput")
        level = 5

    # ------------------------------------------------------- DRAM internals
    cnt_dram = nc.dram_tensor("cnt_dram", [SH, VPADD], F32)
    ag_in = [nc.dram_tensor(f"ag_in{t}", [V * K], F32) for t in range(SH)]
    ag_out = [nc.dram_tensor(f"ag_out{t}", [ROWS_L * K], F32,
                             addr_space="Shared") for t in range(SH)]

    from contextlib import ExitStack
    with tile.TileContext(nc) as tc, ExitStack() as ctx:
        singles = ctx.enter_context(tc.tile_pool(name="singles", bufs=1))
        xpool = ctx.enter_context(tc.tile_pool(name="xpool", bufs=2))
        cntp = ctx.enter_context(tc.tile_pool(name="cntp", bufs=2))
        prodp = ctx.enter_context(tc.tile_pool(name="prodp", bufs=2))
        hb = ctx.enter_context(tc.tile_pool(name="hb", bufs=2))
        statp = ctx.enter_context(tc.tile_pool(name="statp", bufs=3))
        lp = ctx.enter_context(tc.tile_pool(name="lp", bufs=2))
        gp = ctx.enter_context(tc.tile_pool(name="gp", bufs=2))
        ph = ctx.enter_context(tc.tile_pool(name="ph", bufs=3, space="PSUM"))
        pl = ctx.enter_context(tc.tile_pool(name="pl", bufs=2, space="PSUM"))

        # ------------------------------------------------------ load inputs
        et_sb = singles.tile([P, TD, V], F32)         # E^T  [p, td, v]
        nc.sync.dma_start(
            out=et_sb[:],
            in_=AP(tensor=emb_t, offset=0, ap=[[V, P], [P * V, TD], [1, V]]))

        g_sb = singles.tile([P, 3 * SH, TD], F32)
        nc.sync.dma_start(
            out=g_sb[:],
            in_=AP(tensor=gvec, offset=0,
                   ap=[[TD, P], [P * TD, 3 * SH], [1, TD]]))
        b_sb = singles.tile([P, 3 * SH, TD], F32)
        nc.sync.dma_start(
            out=b_sb[:],
            in_=AP(tensor=bvec, offset=0,
                   ap=[[TD, P], [P * TD, 3 * SH], [1, TD]]))

        lw_sb = singles.tile([P, TD, K], F32)
        nc.sync.dma_start(out=lw_sb[:], in_=linw_t.ap())
        lws_sb = singles.tile([P, TD, K], F32)
        nc.scalar.mul(lws_sb[:], lw_sb[:], 1.0 / S)   # fold mean-pool 1/S

        lb_sb = singles.tile([P, K], F32)
        nc.sync.dma_start(out=lb_sb[:], in_=linb_r.ap())

        hb_sb = singles.tile([P, SH * TN], BF16)
        nc.sync.dma_start(out=hb_sb[:], in_=hist_b.ap())
        ha_sb = singles.tile([P, SH * TN], BF16)
        nc.sync.dma_start(out=ha_sb[:], in_=hist_a.ap())
        io_sb = singles.tile([P, P], BF16)
        nc.sync.dma_start(out=io_sb[:], in_=iota128.ap())
        ar_sb = singles.tile([P, VA], BF16)
        nc.sync.dma_start(out=ar_sb[:], in_=arow.ap())
        ho_sb = singles.tile([P, HOFF_COLS], mybir.dt.int32)
        nc.sync.dma_start(out=ho_sb[:], in_=hoff.ap())

        wsum = singles.tile([P, SH, NB, K], F32)      # per-wave gather sums
        if level < 5:
            nc.vector.memset(wsum[:], 0.0)
        eps_sb = singles.tile([P, 1], F32)
        nc.vector.memset(eps_sb[:], EPS)

        # ---------------------------------------------------- histogram (PE)
        cnt_rep = []   # per-slot CNT replicated across partitions [P, V]
        for sl in range(SH):
            psum_h = ph.tile([P, VA], F32, space="PSUM")
            base = sl * TN
            done = 0
            while done < TN:
                gsz = min(GB, TN - done)
                oh = hb.tile([P, GB, P], BF16, tag="oh")
                nc.vector.tensor_tensor(
                    out=oh[:, :gsz, :],
                    in0=hb_sb[:, base + done:base + done + gsz]
                        .unsqueeze(2).to_broadcast([P, gsz, P]),
                    in1=io_sb[:].unsqueeze(1).to_broadcast([P, gsz, P]),
                    op=mybir.AluOpType.is_equal)
                mk = hb.tile([P, GB, VA], BF16, tag="mk")
                nc.vector.tensor_tensor(
                    out=mk[:, :gsz, :],
                    in0=ha_sb[:, base + done:base + done + gsz]
                        .unsqueeze(2).to_broadcast([P, gsz, VA]),
                    in1=ar_sb[:].unsqueeze(1).to_broadcast([P, gsz, VA]),
                    op=mybir.AluOpType.is_equal)
                for g in range(gsz):
                    nc.tensor.matmul(
                        out=psum_h[:],
                        lhsT=oh[:, g, :],
                        rhs=mk[:, g, :],
                        start=(done + g == 0),
                        stop=(done + g == TN - 1))
                done += gsz
            cnt_sb = statp.tile([P, VA], F32, tag="cnt_sb")
            nc.scalar.copy(cnt_sb[:], psum_h[:])
            # scatter to flat per-slot histogram: cnt_dram[sl, a*128 + b]
            nc.sync.dma_start(
                out=AP(tensor=cnt_dram, offset=sl * VPADD,
                       ap=[[1, P], [P, VA]]),
                in_=cnt_sb[:])
            rep = cntp.tile([P, V], F32, tag="cntrep")
            nc.sync.dma_start(
                out=rep[:],
                in_=AP(tensor=cnt_dram, offset=sl * VPADD,
                       ap=[[0, P], [1, V]]))
            cnt_rep.append(rep)
            if debug:
                nc.sync.dma_start(
                    out=AP(tensor=dbg_cnt, offset=sl * VPADD,
                           ap=[[1, P], [P, VA]]),
                    in_=cnt_sb[:])

        # ------------------------------------------- per-slot BN-table chain
        for sl in range(SH if level >= 2 else 0):
            rep = cnt_rep[sl]
            x_cur = et_sb
            for stage in range(3):
                ssum = statp.tile([P, TD], F32, tag="ssum")
                ssq = statp.tile([P, TD], F32, tag="ssq")
                for td in range(TD):
                    prod = prodp.tile([P, V], F32, tag="prod")
                    nc.vector.tensor_mul(prod[:], x_cur[:, td, :], rep[:])
                    nc.vector.tensor_reduce(
                        out=ssum[:, td:td + 1], in_=prod[:],
                        axis=mybir.AxisListType.X, op=mybir.AluOpType.add)
                    prod2 = prodp.tile([P, V], F32, tag="prod2")
                    nc.vector.tensor_mul(prod2[:], prod[:], x_cur[:, td, :])
                    nc.vector.tensor_reduce(
                        out=ssq[:, td:td + 1], in_=prod2[:],
                        axis=mybir.AxisListType.X, op=mybir.AluOpType.add)
                if level < 2.2:
                    continue
                # stats -> affine params A, B  (all [P, TD])
                mu = statp.tile([P, TD], F32, tag="mu")
                nc.vector.tensor_scalar_mul(mu[:], ssum[:], 1.0 / N)
                e2 = statp.tile([P, TD], F32, tag="e2")
                nc.vector.tensor_scalar_mul(e2[:], ssq[:], 1.0 / N)
                mu2 = statp.tile([P, TD], F32, tag="mu2")
                nc.vector.tensor_mul(mu2[:], mu[:], mu[:])
                var = statp.tile([P, TD], F32, tag="var")
                nc.vector.tensor_sub(var[:], e2[:], mu2[:])
                sd = statp.tile([P, TD], F32, tag="sd")
                nc.scalar.activation(sd[:], var[:],
                                     mybir.ActivationFunctionType.Sqrt,
                                     bias=eps_sb[:])
                rinv = statp.tile([P, TD], F32, tag="rinv")
                nc.vector.reciprocal(rinv[:], sd[:])
                # ACT scale/bias operands need 64B-aligned offsets -> pad to
                # 16-f32 slots per td.
                aff_a = statp.tile([P, TD, 16], F32, tag="aff_a")
                nc.vector.tensor_mul(aff_a[:, :, 0:1], rinv[:].unsqueeze(2),
                                     g_sb[:, stage * SH + sl, :].unsqueeze(2))
                mua = statp.tile([P, TD], F32, tag="mua")
                nc.vector.tensor_mul(mua[:], mu[:], aff_a[:, :, 0])
                aff_b = statp.tile([P, TD, 16], F32, tag="aff_b")
                nc.vector.tensor_sub(
                    aff_b[:, :, 0:1],
                    b_sb[:, stage * SH + sl, :].unsqueeze(2),
                    mua[:].unsqueeze(2))
                if level < 2.4:
                    continue
                # x_next = relu(A * x + B)
                x_next = xpool.tile([P, TD, V], F32, tag="x")
                for td in range(TD):
                    nc.scalar.activation(
                        x_next[:, td, :], x_cur[:, td, :],
                        mybir.ActivationFunctionType.Relu,
                        bias=aff_b[:, td, 0:1], scale=aff_a[:, td, 0:1])
                x_cur = x_next

            if level < 2.5:
                continue
            # ------------------------------- L_s = x4 @ (W.T / S)  [V, K]
            l_sb = lp.tile([P, VA, K], F32, tag="lsb")
            for vt in range(VA):
                vp = min(P, V - vt * P)
                psum_l = pl.tile([P, K], F32, space="PSUM", tag="psl")
                for td in range(TD):
                    nc.tensor.matmul(
                        out=psum_l[:vp, :],
                        lhsT=x_cur[:, td, vt * P:vt * P + vp],
                        rhs=lws_sb[:, td, :],
                        start=(td == 0), stop=(td == TD - 1))
                nc.scalar.copy(l_sb[:vp, vt, :], psum_l[:vp, :])
            if level < 3:
                continue
            # publish: ag_in[sl][(vt*128+p)*K + k] = l_sb[p, vt, k]
            nc.sync.dma_start(
                out=AP(tensor=ag_in[sl], offset=0,
                       ap=[[K, P], [P * K, VA - 1], [1, K]]),
                in_=l_sb[:, 0:VA - 1, :])
            vtail = V - (VA - 1) * P      # 25
            nc.sync.dma_start(
                out=AP(tensor=ag_in[sl], offset=(VA - 1) * P * K,
                       ap=[[K, vtail], [1, K]]),
                in_=l_sb[:vtail, VA - 1, :])

            # --------------------------- wave collective + gather + reduce
            nc.gpsimd.collective_compute(
                "AllGather",
                mybir.AluOpType.bypass,
                replica_groups=[list(range(NCORES))],
                ins=[ag_in[sl].ap()],
                outs=[ag_out[sl].ap()],
            )
            if debug:
                nc.sync.dma_start(
                    out=AP(tensor=dbg_l, offset=sl * ROWS_L * K,
                           ap=[[1, ROWS_L * K]]),
                    in_=AP(tensor=ag_out[sl], offset=0, ap=[[1, ROWS_L * K]]))
            if level < 5:
                continue
            # gather L rows (7 f32 each) for this core's nodes via indirect
            # DMA: out[p, m, :] = ag_out[sl][off[p, m]*7 : +7],
            # m = sw + w*nb  (sw = s - 8*sl within the wave)
            w = WAVE_W[sl]
            nbspan = WAVE_NBSPAN[sl]
            wave_base = NB * sum(WAVE_W[t] for t in range(sl))
            for ch in range(WAVE_CHUNKS[sl]):
                cols = nbspan * w                     # 80
                gt = gp.tile([P, cols, K], F32, tag="gt")
                # HW indirect DMA handles exactly one offset per partition
                # per call -> one call per (position, node-tile)
                for m in range(cols):
                    col = wave_base + ch * cols + m
                    nc.gpsimd.indirect_dma_start(
                        out=gt[:, m, :],
                        out_offset=None,
                        in_=AP(tensor=ag_out[sl], offset=0,
                               ap=[[K, ROWS_L], [1, K]]),
                        in_offset=bass.IndirectOffsetOnAxis(
                            ap=ho_sb[:, col:col + 1],
                            axis=0))
                # sum over the w gathered positions -> wsum[sl]
                gv = gt[:].rearrange("p (nb sw) e -> p nb e sw", sw=w)
                nc.vector.tensor_reduce(
                    out=wsum[:, sl, ch * nbspan:(ch + 1) * nbspan, :],
                    in_=gv[:],
                    axis=mybir.AxisListType.X,
                    op=mybir.AluOpType.add)

        # ------------------------------------------- logits + log_softmax
        acc = singles.tile([P, NB, K], F32)
        nc.vector.tensor_add(acc[:], wsum[:, 0], wsum[:, 1])
        nc.vector.tensor_add(acc[:], acc[:], wsum[:, 2])
        nc.vector.tensor_add(acc[:], acc[:],
                             lb_sb[:].unsqueeze(1).to_broadcast([P, NB, K]))
        mx = singles.tile([P, NB], F32)
        nc.vector.tensor_reduce(out=mx[:], in_=acc[:],
                                axis=mybir.AxisListType.X,
                                op=mybir.AluOpType.max)
        xm = singles.tile([P, NB, K], F32)
        nc.vector.tensor_sub(xm[:], acc[:],
                             mx[:].unsqueeze(2).to_broadcast([P, NB, K]))
        ex = singles.tile([P, NB, K], F32)
        nc.scalar.activation(ex[:], xm[:], mybir.ActivationFunctionType.Exp)
        se = singles.tile([P, NB], F32)
        nc.vector.tensor_reduce(out=se[:], in_=ex[:],
                                axis=mybir.AxisListType.X,
                                op=mybir.AluOpType.add)
        ls = singles.tile([P, NB], F32)
        nc.scalar.activation(ls[:], se[:], mybir.ActivationFunctionType.Ln)
        res = singles.tile([P, NB, K], F32)
        nc.vector.tensor_sub(res[:], xm[:],
                             ls[:].unsqueeze(2).to_broadcast([P, NB, K]))

        # ------------------------------------------------------- output DMA
        # node n = nb*128 + p -> out row n (n < 2500)
        nc.sync.dma_start(
            out=AP(tensor=out_ext, offset=0,
                   ap=[[K, P], [P * K, NB - 1], [1, K]]),
            in_=res[:, 0:NB - 1, :])
        tail = NLOC - (NB - 1) * P    # 68
        nc.sync.dma_start(
            out=AP(tensor=out_ext, offset=(NB - 1) * P * K,
                   ap=[[K, tail], [1, K]]),
            in_=res[:tail, NB - 1, :])

    nc.compile()
    return nc


def _host_prep(inputs):
    """Pure layout marshalling of the (numpy) inputs into per-core maps."""
    idx = np.asarray(inputs["sampled_idx"], dtype=np.int32)
    E = np.asarray(inputs["emb_table"], dtype=np.float32)
    lin_w = np.asarray(inputs["lin_w"], dtype=np.float32)
    lin_b = np.asarray(inputs["lin_b"], dtype=np.float32)
    gs = [np.asarray(inputs[f"g{i}"], np.float32).reshape(S, D) for i in (1, 2, 3)]
    bs = [np.asarray(inputs[f"b{i}"], np.float32).reshape(S, D) for i in (1, 2, 3)]

    emb_t = np.ascontiguousarray(E.T)                       # [D, V]
    # lin_w.T arranged [p, td, k]
    lwt = lin_w.T.reshape(TD, P, K).transpose(1, 0, 2)      # [P, TD, K]
    linw_t = np.ascontiguousarray(lwt)
    linb_r = np.tile(lin_b[None, :], (P, 1))                # [P, K]
    iota128 = np.tile(np.arange(P, dtype=np.float32)[None, :], (P, 1)) \
        .astype(ml_dtypes.bfloat16)
    arow = np.tile(np.arange(VA, dtype=np.float32)[None, :], (P, 1)) \
        .astype(ml_dtypes.bfloat16)

    in_maps = []
    for c in range(NCORES):
        slots = [c, c + 8, c + 16 if c < 4 else -1]
        # g/b per (stage, slot): value g[s, td*128+p] laid out [P, TD]
        gv = np.zeros((3 * SH, P, TD), np.float32)
        bv = np.zeros((3 * SH, P, TD), np.float32)
        for st in range(3):
            for sl, s in enumerate(slots):
                if s < 0:
                    continue
                gv[st * SH + sl] = gs[st][s].reshape(TD, P).T
                bv[st * SH + sl] = bs[st][s].reshape(TD, P).T
        # hist columns
        hb_arr = np.full((P, SH * TN), -1.0, np.float32)
        ha_arr = np.full((P, SH * TN), -1.0, np.float32)
        for sl, s in enumerate(slots):
            if s < 0:
                continue
            col = np.full(NH, -1, np.int32)
            col[:N] = idx[:, s]
            cb = np.where(col >= 0, col % P, -1).astype(np.float32)
            ca = np.where(col >= 0, col // P, -1).astype(np.float32)
            hb_arr[:, sl * TN:(sl + 1) * TN] = cb.reshape(TN, P).T
            ha_arr[:, sl * TN:(sl + 1) * TN] = ca.reshape(TN, P).T

        # gather offsets [p, m]: m = sw + w*nb per wave, value = row of the
        # gathered L table = (s - 8*wave)*V + idx[n, s], n = nb*128 + p
        idx_shard = idx[c * NLOC:(c + 1) * NLOC]            # [2500, S]
        gcols = []
        for t in range(SH):
            w = WAVE_W[t]
            m = np.arange(NB * w)
            sw = m % w
            nb = m // w
            block = np.zeros((P, NB * w), np.int32)
            for p in range(P):
                n = nb * P + p
                s_glob = 8 * t + sw
                v = np.where(n < NLOC,
                             sw * V + idx_shard[np.minimum(n, NLOC - 1), s_glob],
                             0)
                block[p] = v
            gcols.append(block)
        hoff_arr = np.concatenate(gcols, axis=1).astype(np.int32)  # [128, 400]

        in_map = {
            "emb_t": emb_t,
            "gvec": gv,
            "bvec": bv,
            "linw_t": linw_t,
            "linb_r": linb_r,
            "hist_b": hb_arr.astype(ml_dtypes.bfloat16),
            "hist_a": ha_arr.astype(ml_dtypes.bfloat16),
            "iota128": iota128,
            "arow": arow,
            "hoff": hoff_arr,
        }
        in_maps.append(in_map)
    return in_maps


_NC_CACHE = {}


def _get_nc():
    if "nc" not in _NC_CACHE:
        _NC_CACHE["nc"] = _build()
    return _NC_CACHE["nc"]


def _get_runner():
    """Cached jitted SPMD executor (modeled on bass2jax.run_bass_via_pjrt,
    without buffer donation so the same device buffers can be re-executed
    for timing). Returns (fn, pack, unpack)."""
    if "runner" in _NC_CACHE:
        return _NC_CACHE["runner"]
    import jax
    from jax.sharding import Mesh, PartitionSpec
    from jax.experimental.shard_map import shard_map
    from concourse import bass2jax

    nc = _get_nc()
    bass2jax.install_neuronx_cc_hook()

    in_names, out_names, out_avals, zero_outs = [], [], [], []
    partition_name = (nc.partition_id_tensor.name
                      if nc.partition_id_tensor else None)
    for alloc in nc.m.functions[0].allocations:
        if not isinstance(alloc, mybir.MemoryLocationSet):
            continue
        name = alloc.memorylocations[0].name
        if alloc.kind == "ExternalInput":
            if name != partition_name:
                in_names.append(name)
        elif alloc.kind == "ExternalOutput":
            out_names.append(name)
            shape = tuple(alloc.tensor_shape)
            dtype = mybir.dt.np(alloc.dtype)
            out_avals.append(jax.core.ShapedArray(shape, dtype))
            zero_outs.append(np.zeros(shape, dtype))
    n_params = len(in_names)
    all_names = in_names + out_names
    if partition_name is not None:
        all_names.append(partition_name)

    def _body(*args):
        operands = list(args)
        if partition_name is not None:
            operands.append(bass2jax.partition_id_tensor())
        outs = bass2jax._bass_exec_p.bind(
            *operands,
            out_avals=tuple(out_avals),
            in_names=tuple(all_names),
            out_names=tuple(out_names),
            lowering_input_output_aliases=(),
            sim_require_finite=True,
            sim_require_nnan=True,
            nc=nc,
        )
        return tuple(outs)

    devices = jax.devices()[:NCORES]
    mesh = Mesh(np.asarray(devices), ("core",))
    n_outs = len(out_names)
    sharded = jax.jit(
        shard_map(_body, mesh=mesh,
                  in_specs=(PartitionSpec("core"),) * (n_params + n_outs),
                  out_specs=(PartitionSpec("core"),) * n_outs,
                  check_rep=False),
        keep_unused=True)

    def pack(in_maps):
        concat_in = [
            np.concatenate([np.asarray(in_maps[c][name])
                            for c in range(NCORES)], axis=0)
            for name in in_names
        ]
        concat_zeros = [
            np.zeros((NCORES * z.shape[0], *z.shape[1:]), z.dtype)
            for z in zero_outs
        ]
        return [jax.device_put(a) for a in concat_in + concat_zeros]

    def unpack(out_arrs):
        res = np.asarray(out_arrs[out_names.index("out")])
        return res.reshape(NCORES, NLOC, K)

    _NC_CACHE["runner"] = (sharded, pack, unpack)
    return _NC_CACHE["runner"]


def kernel(**inputs):
    fn, pack, unpack = _get_runner()
    args = pack(_host_prep(inputs))
    shards = unpack(fn(*args))
    return np.concatenate(list(shards), axis=0)


# revision 38
# speedup vs baseline: 1332.2131x; 1332.2131x over previous
"""Trainium2 Bass kernel for nn_AMPGCN (embedding_lookup + 3x BatchNorm/ReLU +
mean-pool + linear + log_softmax), distributed over 8 NeuronCores.

Algorithm
---------
Every activation x[n, (s,d)] through the BN/ReLU chain depends only on
(v=idx[n,s], s, d), because BatchNorm is a per-channel affine map. So instead
of materializing the [20000, 15360] activation tensor (1.2 GB), each core:

  1. builds per-position histograms CNT[s, v] of the sampled indices
     (one-hot outer-product matmuls on the TensorEngine),
  2. runs the BN chain on tiny per-position tables T_s [1433, 768]
     (weighted sums with CNT give the exact batch statistics),
  3. collapses the linear head into per-position tables L_s = T_s @ W.T / S,
  4. all-gathers the L tables and gathers 7-float rows per (node, s)
     via dma_gather, sums over s, and applies log_softmax.

Work is sharded over the position axis s (20 positions over 8 cores, 3 waves),
the final gather + softmax over the node axis (2500 nodes/core).
"""
import sys

if "/opt/trn_rl_repo" not in sys.path:
    sys.path.insert(0, "/opt/trn_rl_repo")

import numpy as np
import ml_dtypes

import concourse.bass as bass
import concourse.tile as tile
from concourse import bacc, mybir
from concourse.bass import AP

# ---------------------------------------------------------------- constants
N, S, D, V, K = 20000, 20, 768, 1433, 7
EPS = 1e-5
NCORES = 8
P = 128
NLOC = N // NCORES            # 2500 nodes per core
NB = 20                       # node tiles per core (2560 padded)
NPAD = NB * P                 # 2560
TD = D // P                   # 6 d-tiles
SH = 3                        # s-slots per core (cores 4-7: slot 2 is dummy)
TN = 157                      # hist row-tiles per slot (ceil(20000/128))
NH = TN * P                   # 20096 padded hist rows
VA = 12                       # a = idx // 128 in [0, 12)
VPADD = 1536                  # VA * 128, padded vocab per slot in cnt scratch
GB = 8                        # hist batch size (tiles per vector instr)
ROWS_L = NCORES * V           # 11464 rows per gathered-wave table
EPITCH = 64                   # gather row pitch (64 f32 = 256B)
F32 = mybir.dt.float32
BF16 = mybir.dt.bfloat16
I16 = mybir.dt.int16

# wave t gathers positions s = 8t + c (c = contributing core); widths:
WAVE_W = [8, 8, 4]
WAVE_NBSPAN = [10, 10, 20]    # node-tile span per gather chunk
WAVE_CHUNKS = [2, 2, 1]
HOFF_COLS = NB * sum(WAVE_W)  # 400 gather offsets per partition


def _build(level=5, reps=1):
    """Build + compile the SPMD Bass graph (identical on all 8 cores).

    level: pipeline prefix for hardware bisection
      1 = loads + histogram, 2 = +BN chain +L, 3 = +publish +collective,
      4 = +repad, 5 = full (gather + softmax).
    """
    nc = bacc.Bacc("TRN2", target_bir_lowering=False, debug=False,
                   num_devices=NCORES)

    # ------------------------------------------------------------- params
    emb_t = nc.dram_tensor("emb_t", [D, V], F32, kind="ExternalInput")
    # BN affine params, one [P, TD] block per (stage, slot)
    gvec = nc.dram_tensor("gvec", [3 * SH, P, TD], F32, kind="ExternalInput")
    bvec = nc.dram_tensor("bvec", [3 * SH, P, TD], F32, kind="ExternalInput")
    linw_t = nc.dram_tensor("linw_t", [P, TD, K], F32, kind="ExternalInput")
    linb_r = nc.dram_tensor("linb_r", [P, K], F32, kind="ExternalInput")
    hist_b = nc.dram_tensor("hist_b", [P, SH * TN], BF16, kind="ExternalInput")
    hist_a = nc.dram_tensor("hist_a", [P, SH * TN], BF16, kind="ExternalInput")
    iota128 = nc.dram_tensor("iota128", [P, P], BF16, kind="ExternalInput")
    arow = nc.dram_tensor("arow", [P, VA], BF16, kind="ExternalInput")
    hoff = nc.dram_tensor("hoff", [P, HOFF_COLS], mybir.dt.int32,
                          kind="ExternalInput")
    out_ext = nc.dram_tensor("out", [NLOC, K], F32, kind="ExternalOutput")
    debug = level == 5.1
    if debug:
        dbg_cnt = nc.dram_tensor("dbg_cnt", [SH, VPADD], F32,
                                 kind="ExternalOutput")
        dbg_l = nc.dram_tensor("dbg_l", [SH, ROWS_L * K], F32,
                               kind="ExternalOutput")
        level = 5

    # ------------------------------------------------------- DRAM internals
    cnt_dram = nc.dram_tensor("cnt_dram", [SH, VPADD], F32)
    ag_in = [nc.dram_tensor(f"ag_in{t}", [V * K], F32) for t in range(SH)]
    ag_out = [nc.dram_tensor(f"ag_out{t}", [ROWS_L * K], F32,
                             addr_space="Shared") for t in range(SH)]

    from contextlib import ExitStack
    with tile.TileContext(nc) as tc, ExitStack() as ctx:
      singles = ctx.enter_context(tc.tile_pool(name="singles", bufs=1))
      xpool = ctx.enter_context(tc.tile_pool(name="xpool", bufs=2))
      cntp = ctx.enter_context(tc.tile_pool(name="cntp", bufs=2))
      prodp = ctx.enter_context(tc.tile_pool(name="prodp", bufs=2))
      hb = ctx.enter_context(tc.tile_pool(name="hb", bufs=2))
      statp = ctx.enter_context(tc.tile_pool(name="statp", bufs=3))
      lp = ctx.enter_context(tc.tile_pool(name="lp", bufs=2))
      gp = ctx.enter_context(tc.tile_pool(name="gp", bufs=2))
      ph = ctx.enter_context(tc.tile_pool(name="ph", bufs=3, space="PSUM"))
      pl = ctx.enter_context(tc.tile_pool(name="pl", bufs=2, space="PSUM"))
      for _rep in range(reps):

        # ------------------------------------------------------ load inputs
        et_sb = singles.tile([P, TD, V], F32)         # E^T  [p, td, v]
        nc.sync.dma_start(
            out=et_sb[:],
            in_=AP(tensor=emb_t, offset=0, ap=[[V, P], [P * V, TD], [1, V]]))

        g_sb = singles.tile([P, 3 * SH, TD], F32)
        nc.sync.dma_start(
            out=g_sb[:],
            in_=AP(tensor=gvec, offset=0,
                   ap=[[TD, P], [P * TD, 3 * SH], [1, TD]]))
        b_sb = singles.tile([P, 3 * SH, TD], F32)
        nc.sync.dma_start(
            out=b_sb[:],
            in_=AP(tensor=bvec, offset=0,
                   ap=[[TD, P], [P * TD, 3 * SH], [1, TD]]))

        lw_sb = singles.tile([P, TD, K], F32)
        nc.sync.dma_start(out=lw_sb[:], in_=linw_t.ap())
        lws_sb = singles.tile([P, TD, K], F32)
        nc.scalar.mul(lws_sb[:], lw_sb[:], 1.0 / S)   # fold mean-pool 1/S

        lb_sb = singles.tile([P, K], F32)
        nc.sync.dma_start(out=lb_sb[:], in_=linb_r.ap())

        hb_sb = singles.tile([P, SH * TN], BF16)
        nc.sync.dma_start(out=hb_sb[:], in_=hist_b.ap())
        ha_sb = singles.tile([P, SH * TN], BF16)
        nc.sync.dma_start(out=ha_sb[:], in_=hist_a.ap())
        io_sb = singles.tile([P, P], BF16)
        nc.sync.dma_start(out=io_sb[:], in_=iota128.ap())
        ar_sb = singles.tile([P, VA], BF16)
        nc.sync.dma_start(out=ar_sb[:], in_=arow.ap())
        ho_sb = singles.tile([P, HOFF_COLS], mybir.dt.int32)
        nc.sync.dma_start(out=ho_sb[:], in_=hoff.ap())

        wsum = singles.tile([P, SH, NB, K], F32)      # per-wave gather sums
        if level < 5:
            nc.vector.memset(wsum[:], 0.0)
        eps_sb = singles.tile([P, 1], F32)
        nc.vector.memset(eps_sb[:], EPS)

        # ---------------------------------------------------- histogram (PE)
        cnt_rep = []   # per-slot CNT replicated across partitions [P, V]
        for sl in range(SH):
            psum_h = ph.tile([P, VA], F32, space="PSUM")
            base = sl * TN
            done = 0
            while done < TN:
                gsz = min(GB, TN - done)
                oh = hb.tile([P, GB, P], BF16, tag="oh")
                nc.vector.tensor_tensor(
                    out=oh[:, :gsz, :],
                    in0=hb_sb[:, base + done:base + done + gsz]
                        .unsqueeze(2).to_broadcast([P, gsz, P]),
                    in1=io_sb[:].unsqueeze(1).to_broadcast([P, gsz, P]),
                    op=mybir.AluOpType.is_equal)
                mk = hb.tile([P, GB, VA], BF16, tag="mk")
                nc.vector.tensor_tensor(
                    out=mk[:, :gsz, :],
                    in0=ha_sb[:, base + done:base + done + gsz]
                        .unsqueeze(2).to_broadcast([P, gsz, VA]),
                    in1=ar_sb[:].unsqueeze(1).to_broadcast([P, gsz, VA]),
                    op=mybir.AluOpType.is_equal)
                for g in range(gsz):
                    nc.tensor.matmul(
                        out=psum_h[:],
                        lhsT=oh[:, g, :],
                        rhs=mk[:, g, :],
                        start=(done + g == 0),
                        stop=(done + g == TN - 1))
                done += gsz
            cnt_sb = statp.tile([P, VA], F32, tag="cnt_sb")
            nc.scalar.copy(cnt_sb[:], psum_h[:])
            # scatter to flat per-slot histogram: cnt_dram[sl, a*128 + b]
            nc.sync.dma_start(
                out=AP(tensor=cnt_dram, offset=sl * VPADD,
                       ap=[[1, P], [P, VA]]),
                in_=cnt_sb[:])
            rep = cntp.tile([P, V], F32, tag="cntrep")
            nc.sync.dma_start(
                out=rep[:],
                in_=AP(tensor=cnt_dram, offset=sl * VPADD,
                       ap=[[0, P], [1, V]]))
            cnt_rep.append(rep)
            if debug:
                nc.sync.dma_start(
                    out=AP(tensor=dbg_cnt, offset=sl * VPADD,
                           ap=[[1, P], [P, VA]]),
                    in_=cnt_sb[:])

        # ------------------------------------------- per-slot BN-table chain
        for sl in range(SH if level >= 2 else 0):
            rep = cnt_rep[sl]
            x_cur = et_sb
            for stage in range(3):
                ssum = statp.tile([P, TD], F32, tag="ssum")
                ssq = statp.tile([P, TD], F32, tag="ssq")
                for td in range(TD):
                    prod = prodp.tile([P, V], F32, tag="prod")
                    nc.vector.tensor_mul(prod[:], x_cur[:, td, :], rep[:])
                    nc.vector.tensor_reduce(
                        out=ssum[:, td:td + 1], in_=prod[:],
                        axis=mybir.AxisListType.X, op=mybir.AluOpType.add)
                    prod2 = prodp.tile([P, V], F32, tag="prod2")
                    nc.vector.tensor_mul(prod2[:], prod[:], x_cur[:, td, :])
                    nc.vector.tensor_reduce(
                        out=ssq[:, td:td + 1], in_=prod2[:],
                        axis=mybir.AxisListType.X, op=mybir.AluOpType.add)
                if level < 2.2:
                    continue
                # stats -> affine params A, B  (all [P, TD])
                mu = statp.tile([P, TD], F32, tag="mu")
                nc.vector.tensor_scalar_mul(mu[:], ssum[:], 1.0 / N)
                e2 = statp.tile([P, TD], F32, tag="e2")
                nc.vector.tensor_scalar_mul(e2[:], ssq[:], 1.0 / N)
                mu2 = statp.tile([P, TD], F32, tag="mu2")
                nc.vector.tensor_mul(mu2[:], mu[:], mu[:])
                var = statp.tile([P, TD], F32, tag="var")
                nc.vector.tensor_sub(var[:], e2[:], mu2[:])
                sd = statp.tile([P, TD], F32, tag="sd")
                nc.scalar.activation(sd[:], var[:],
                                     mybir.ActivationFunctionType.Sqrt,
                                     bias=eps_sb[:])
                rinv = statp.tile([P, TD], F32, tag="rinv")
                nc.vector.reciprocal(rinv[:], sd[:])
                # ACT scale/bias operands need 64B-aligned offsets -> pad to
                # 16-f32 slots per td.
                aff_a = statp.tile([P, TD, 16], F32, tag="aff_a")
                nc.vector.tensor_mul(aff_a[:, :, 0:1], rinv[:].unsqueeze(2),
                                     g_sb[:, stage * SH + sl, :].unsqueeze(2))
                mua = statp.tile([P, TD], F32, tag="mua")
                nc.vector.tensor_mul(mua[:], mu[:], aff_a[:, :, 0])
                aff_b = statp.tile([P, TD, 16], F32, tag="aff_b")
                nc.vector.tensor_sub(
                    aff_b[:, :, 0:1],
                    b_sb[:, stage * SH + sl, :].unsqueeze(2),
                    mua[:].unsqueeze(2))
                if level < 2.4:
                    continue
                # x_next = relu(A * x + B)
                x_next = xpool.tile([P, TD, V], F32, tag="x")
                for td in range(TD):
                    nc.scalar.activation(
                        x_next[:, td, :], x_cur[:, td, :],
                        mybir.ActivationFunctionType.Relu,
                        bias=aff_b[:, td, 0:1], scale=aff_a[:, td, 0:1])
                x_cur = x_next

            if level < 2.5:
                continue
            # ------------------------------- L_s = x4 @ (W.T / S)  [V, K]
            l_sb = lp.tile([P, VA, K], F32, tag="lsb")
            for vt in range(VA):
                vp = min(P, V - vt * P)
                psum_l = pl.tile([P, K], F32, space="PSUM", tag="psl")
                for td in range(TD):
                    nc.tensor.matmul(
                        out=psum_l[:vp, :],
                        lhsT=x_cur[:, td, vt * P:vt * P + vp],
                        rhs=lws_sb[:, td, :],
                        start=(td == 0), stop=(td == TD - 1))
                nc.scalar.copy(l_sb[:vp, vt, :], psum_l[:vp, :])
            if level < 3:
                continue
            # publish: ag_in[sl][(vt*128+p)*K + k] = l_sb[p, vt, k]
            nc.sync.dma_start(
                out=AP(tensor=ag_in[sl], offset=0,
                       ap=[[K, P], [P * K, VA - 1], [1, K]]),
                in_=l_sb[:, 0:VA - 1, :])
            vtail = V - (VA - 1) * P      # 25
            nc.sync.dma_start(
                out=AP(tensor=ag_in[sl], offset=(VA - 1) * P * K,
                       ap=[[K, vtail], [1, K]]),
                in_=l_sb[:vtail, VA - 1, :])

            # --------------------------- wave collective + gather + reduce
            nc.gpsimd.collective_compute(
                "AllGather",
                mybir.AluOpType.bypass,
                replica_groups=[list(range(NCORES))],
                ins=[ag_in[sl].ap()],
                outs=[ag_out[sl].ap()],
            )
            if debug:
                nc.sync.dma_start(
                    out=AP(tensor=dbg_l, offset=sl * ROWS_L * K,
                           ap=[[1, ROWS_L * K]]),
                    in_=AP(tensor=ag_out[sl], offset=0, ap=[[1, ROWS_L * K]]))
            if level < 5:
                continue
            # gather L rows (7 f32 each) for this core's nodes via indirect
            # DMA: out[p, m, :] = ag_out[sl][off[p, m]*7 : +7],
            # m = sw + w*nb  (sw = s - 8*sl within the wave)
            w = WAVE_W[sl]
            nbspan = WAVE_NBSPAN[sl]
            wave_base = NB * sum(WAVE_W[t] for t in range(sl))
            for ch in range(WAVE_CHUNKS[sl]):
                cols = nbspan * w                     # 80
                gt = gp.tile([P, cols, K], F32, tag="gt")
                # HW indirect DMA handles exactly one offset per partition
                # per call -> one call per (position, node-tile)
                for m in range(cols):
                    col = wave_base + ch * cols + m
                    nc.gpsimd.indirect_dma_start(
                        out=gt[:, m, :],
                        out_offset=None,
                        in_=AP(tensor=ag_out[sl], offset=0,
                               ap=[[K, ROWS_L], [1, K]]),
                        in_offset=bass.IndirectOffsetOnAxis(
                            ap=ho_sb[:, col:col + 1],
                            axis=0))
                # sum over the w gathered positions -> wsum[sl]
                gv = gt[:].rearrange("p (nb sw) e -> p nb e sw", sw=w)
                nc.vector.tensor_reduce(
                    out=wsum[:, sl, ch * nbspan:(ch + 1) * nbspan, :],
                    in_=gv[:],
                    axis=mybir.AxisListType.X,
                    op=mybir.AluOpType.add)

        # ------------------------------------------- logits + log_softmax
        acc = singles.tile([P, NB, K], F32)
        nc.vector.tensor_add(acc[:], wsum[:, 0], wsum[:, 1])
        nc.vector.tensor_add(acc[:], acc[:], wsum[:, 2])
        nc.vector.tensor_add(acc[:], acc[:],
                             lb_sb[:].unsqueeze(1).to_broadcast([P, NB, K]))
        mx = singles.tile([P, NB], F32)
        nc.vector.tensor_reduce(out=mx[:], in_=acc[:],
                                axis=mybir.AxisListType.X,
                                op=mybir.AluOpType.max)
        xm = singles.tile([P, NB, K], F32)
        nc.vector.tensor_sub(xm[:], acc[:],
                             mx[:].unsqueeze(2).to_broadcast([P, NB, K]))
        ex = singles.tile([P, NB, K], F32)
        nc.scalar.activation(ex[:], xm[:], mybir.ActivationFunctionType.Exp)
        se = singles.tile([P, NB], F32)
        nc.vector.tensor_reduce(out=se[:], in_=ex[:],
                                axis=mybir.AxisListType.X,
                                op=mybir.AluOpType.add)
        ls = singles.tile([P, NB], F32)
        nc.scalar.activation(ls[:], se[:], mybir.ActivationFunctionType.Ln)
        res = singles.tile([P, NB, K], F32)
        nc.vector.tensor_sub(res[:], xm[:],
                             ls[:].unsqueeze(2).to_broadcast([P, NB, K]))

        # ------------------------------------------------------- output DMA
        # node n = nb*128 + p -> out row n (n < 2500)
        nc.sync.dma_start(
            out=AP(tensor=out_ext, offset=0,
                   ap=[[K, P], [P * K, NB - 1], [1, K]]),
            in_=res[:, 0:NB - 1, :])
        tail = NLOC - (NB - 1) * P    # 68
        nc.sync.dma_start(
            out=AP(tensor=out_ext, offset=(NB - 1) * P * K,
                   ap=[[K, tail], [1, K]]),
            in_=res[:tail, NB - 1, :])

    nc.compile()
    return nc


def _host_prep(inputs):
    """Pure layout marshalling of the (numpy) inputs into per-core maps."""
    idx = np.asarray(inputs["sampled_idx"], dtype=np.int32)
    E = np.asarray(inputs["emb_table"], dtype=np.float32)
    lin_w = np.asarray(inputs["lin_w"], dtype=np.float32)
    lin_b = np.asarray(inputs["lin_b"], dtype=np.float32)
    gs = [np.asarray(inputs[f"g{i}"], np.float32).reshape(S, D) for i in (1, 2, 3)]
    bs = [np.asarray(inputs[f"b{i}"], np.float32).reshape(S, D) for i in (1, 2, 3)]

    emb_t = np.ascontiguousarray(E.T)                       # [D, V]
    # lin_w.T arranged [p, td, k]
    lwt = lin_w.T.reshape(TD, P, K).transpose(1, 0, 2)      # [P, TD, K]
    linw_t = np.ascontiguousarray(lwt)
    linb_r = np.tile(lin_b[None, :], (P, 1))                # [P, K]
    iota128 = np.tile(np.arange(P, dtype=np.float32)[None, :], (P, 1)) \
        .astype(ml_dtypes.bfloat16)
    arow = np.tile(np.arange(VA, dtype=np.float32)[None, :], (P, 1)) \
        .astype(ml_dtypes.bfloat16)

    in_maps = []
    for c in range(NCORES):
        slots = [c, c + 8, c + 16 if c < 4 else -1]
        # g/b per (stage, slot): value g[s, td*128+p] laid out [P, TD]
        gv = np.zeros((3 * SH, P, TD), np.float32)
        bv = np.zeros((3 * SH, P, TD), np.float32)
        for st in range(3):
            for sl, s in enumerate(slots):
                if s < 0:
                    continue
                gv[st * SH + sl] = gs[st][s].reshape(TD, P).T
                bv[st * SH + sl] = bs[st][s].reshape(TD, P).T
        # hist columns
        hb_arr = np.full((P, SH * TN), -1.0, np.float32)
        ha_arr = np.full((P, SH * TN), -1.0, np.float32)
        for sl, s in enumerate(slots):
            if s < 0:
                continue
            col = np.full(NH, -1, np.int32)
            col[:N] = idx[:, s]
            cb = np.where(col >= 0, col % P, -1).astype(np.float32)
            ca = np.where(col >= 0, col // P, -1).astype(np.float32)
            hb_arr[:, sl * TN:(sl + 1) * TN] = cb.reshape(TN, P).T
            ha_arr[:, sl * TN:(sl + 1) * TN] = ca.reshape(TN, P).T

        # gather offsets [p, m]: m = sw + w*nb per wave, value = row of the
        # gathered L table = (s - 8*wave)*V + idx[n, s], n = nb*128 + p
        idx_shard = idx[c * NLOC:(c + 1) * NLOC]            # [2500, S]
        gcols = []
        for t in range(SH):
            w = WAVE_W[t]
            m = np.arange(NB * w)
            sw = m % w
            nb = m // w
            block = np.zeros((P, NB * w), np.int32)
            for p in range(P):
                n = nb * P + p
                s_glob = 8 * t + sw
                v = np.where(n < NLOC,
                             sw * V + idx_shard[np.minimum(n, NLOC - 1), s_glob],
                             0)
                block[p] = v
            gcols.append(block)
        hoff_arr = np.concatenate(gcols, axis=1).astype(np.int32)  # [128, 400]

        in_map = {
            "emb_t": emb_t,
            "gvec": gv,
            "bvec": bv,
            "linw_t": linw_t,
            "linb_r": linb_r,
            "hist_b": hb_arr.astype(ml_dtypes.bfloat16),
            "hist_a": ha_arr.astype(ml_dtypes.bfloat16),
            "iota128": iota128,
            "arow": arow,
            "hoff": hoff_arr,
        }
        in_maps.append(in_map)
    return in_maps


_NC_CACHE = {}


def _get_nc():
    if "nc" not in _NC_CACHE:
        _NC_CACHE["nc"] = _build()
    return _NC_CACHE["nc"]


def _get_runner():
    """Cached jitted SPMD executor (modeled on bass2jax.run_bass_via_pjrt,
    without buffer donation so the same device buffers can be re-executed
    for timing). Returns (fn, pack, unpack)."""
    if "runner" in _NC_CACHE:
        return _NC_CACHE["runner"]
    import jax
    from jax.sharding import Mesh, PartitionSpec
    from jax.experimental.shard_map import shard_map
    from concourse import bass2jax

    nc = _get_nc()
    bass2jax.install_neuronx_cc_hook()

    in_names, out_names, out_avals, zero_outs = [], [], [], []
    partition_name = (nc.partition_id_tensor.name
                      if nc.partition_id_tensor else None)
    for alloc in nc.m.functions[0].allocations:
        if not isinstance(alloc, mybir.MemoryLocationSet):
            continue
        name = alloc.memorylocations[0].name
        if alloc.kind == "ExternalInput":
            if name != partition_name:
                in_names.append(name)
        elif alloc.kind == "ExternalOutput":
            out_names.append(name)
            shape = tuple(alloc.tensor_shape)
            dtype = mybir.dt.np(alloc.dtype)
            out_avals.append(jax.core.ShapedArray(shape, dtype))
            zero_outs.append(np.zeros(shape, dtype))
    n_params = len(in_names)
    all_names = in_names + out_names
    if partition_name is not None:
        all_names.append(partition_name)

    def _body(*args):
        operands = list(args)
        if partition_name is not None:
            operands.append(bass2jax.partition_id_tensor())
        outs = bass2jax._bass_exec_p.bind(
            *operands,
            out_avals=tuple(out_avals),
            in_names=tuple(all_names),
            out_names=tuple(out_names),
            lowering_input_output_aliases=(),
            sim_require_finite=True,
            sim_require_nnan=True,
            nc=nc,
        )
        return tuple(outs)

    devices = jax.devices()[:NCORES]
    mesh = Mesh(np.asarray(devices), ("core",))
    n_outs = len(out_names)
    sharded = jax.jit(
        shard_map(_body, mesh=mesh,
                  in_specs=(PartitionSpec("core"),) * (n_params + n_outs),
                  out_specs=(PartitionSpec("core"),) * n_outs,
                  check_rep=False),
        keep_unused=True)

    def pack(in_maps):
        concat_in = [
            np.concatenate([np.asarray(in_maps[c][name])
                            for c in range(NCORES)], axis=0)
            for name in in_names
        ]
        concat_zeros = [
            np.zeros((NCORES * z.shape[0], *z.shape[1:]), z.dtype)
            for z in zero_outs
        ]
        return [jax.device_put(a) for a in concat_in + concat_zeros]

    def unpack(out_arrs):
        res = np.asarray(out_arrs[out_names.index("out")])
        return res.reshape(NCORES, NLOC, K)

    _NC_CACHE["runner"] = (sharded, pack, unpack)
    return _NC_CACHE["runner"]


def kernel(**inputs):
    fn, pack, unpack = _get_runner()
    args = pack(_host_prep(inputs))
    shards = unpack(fn(*args))
    return np.concatenate(list(shards), axis=0)
